# revision 18
# baseline (speedup 1.0000x reference)
"""AnomalyAttention distributed Bass kernel for 8 TRN2 NeuronCores.

Reference computation (n=4096, d=512):
    qkv = x @ W.T                       # [n, d];  Q = K = V = sigma = qkv
    L   = (Q @ K.T) / sqrt(d)           # [n, n]
    S   = softmax(L, axis=0)            # column softmax
    Z   = S @ V                         # [n, d]
    p[i,j]    = |i - j|
    gaussian  = p + |sigma[:,0]|[None,:] * noise      # noise = fixed jax key(42)
    P   = gaussian / gaussian.sum(-1, keepdims=True)  # row normalized
    returns (Z, P)

Sharding: each core owns a 512-row block i_block = [c*512, (c+1)*512).
Logits are built transposed, L.T[j, i_local] (all j on partitions, local i on
free), so the column softmax reduces along the free axis per partition; the
cross-core part of the reduction is a single 16 KiB AllReduce of per-column
partial sums, hidden behind the P-path (prior matrix) work.  Both layouts of
qkv (transposed for the logits lhsT, natural for Z.T = qkv.T @ S.T) are
computed on-chip in bf16 and stay SBUF-resident.  |i-j| comes from an
on-device iota + scalar-engine Abs with per-partition bias; gaussian and its
row-sum are fused DVE ops; the 1/rowsum scale runs on the scalar engine.
"""

import sys

if "/opt/trn_rl_repo" not in sys.path:
    sys.path.insert(0, "/opt/trn_rl_repo")

from contextlib import ExitStack

import ml_dtypes
import numpy as np

import concourse.bass as bass
import concourse.tile as tile
from concourse import bacc, mybir, bass_utils

N = 4096
D = 512
NC = 8
BLK = N // NC          # 512 rows of S / P per core
P = 128                # partitions
F32 = mybir.dt.float32
BF16 = mybir.dt.bfloat16
INV_SQRT_D = 1.0 / np.sqrt(D)

KC = D // P            # 4 contraction chunks of 128
JC = N // P            # 32 j-chunks of 128
NCH = N // 512         # 8 n-chunks of 512
IC = BLK // P          # 4 local i-chunks of 128
HW_ = 1024             # P-path free-dim tile
NH = N // HW_          # free chunks per i-chunk row

_compiled = None


def _build():
    nc = bacc.Bacc("TRN2", target_bir_lowering=False, debug=False, num_devices=NC)

    # Per-core inputs (bf16 except the P-path data).  xT/wT hold the same
    # data on every core; xTs/noise/ioff are per-core shards.
    xT = nc.dram_tensor("xT", [D, N], BF16, kind="ExternalInput").ap()      # x.T
    xTs = nc.dram_tensor("xTs", [D, BLK], BF16, kind="ExternalInput").ap()  # x.T[:, i_block]
    wT = nc.dram_tensor("wT", [D, D], BF16, kind="ExternalInput").ap()      # W.T
    noise = nc.dram_tensor("noise", [BLK, N], F32, kind="ExternalInput").ap()
    ioff = nc.dram_tensor("ioff", [P, 1], F32, kind="ExternalInput").ap()   # c*BLK

    out_zt = nc.dram_tensor("zt", [D, BLK], F32, kind="ExternalOutput").ap()  # Z.T block
    out_p = nc.dram_tensor("p", [BLK, N], F32, kind="ExternalOutput").ap()    # P rows

    with tile.TileContext(nc) as tc, ExitStack() as big:
        sb = big.enter_context(tc.tile_pool(name="sb", bufs=1))
        psum = big.enter_context(tc.tile_pool(name="psum", bufs=4, space="PSUM"))
        dram = big.enter_context(tc.tile_pool(name="dram", bufs=1, space="DRAM"))
        pstream = big.enter_context(tc.tile_pool(name="pstream", bufs=2))

        # bf16 SBUF residents: qkvT (4 x [128, N]) and natural qkv (32 x [128, D])
        qkvT_sb = [
            sb.tile([P, N], BF16, name=f"qkvT{dc}", tag=f"qkvT{dc}") for dc in range(KC)
        ]
        qn_sb = [
            sb.tile([P, D], BF16, name=f"qn{jc}", tag=f"qn{jc}") for jc in range(JC)
        ]
        sigbc = sb.tile([P, N], BF16, name="sigbc", tag="sigbc")
        ones = sb.tile([1, P], BF16, name="ones", tag="ones")
        nc.vector.memset(ones[:], 1.0)
        ioff_sb = sb.tile([P, 1], F32, name="ioff_sb", tag="ioff_sb")
        nc.sync.dma_start(ioff_sb[:], ioff[:])
        pd = sb.tile([P, JC], F32, name="pd", tag="pd")
        rhs_i = []
        expT = []

        with tc.tile_pool(name="phA", bufs=1) as phA:
            # ---- load W.T, x.T slice ------------------------------------
            wT_sb, xTs_sb = [], []
            for kc in range(KC):
                w = phA.tile([P, D], BF16, name=f"wT{kc}", tag=f"wT{kc}")
                nc.sync.dma_start(w[:], wT[kc * P:(kc + 1) * P, :])
                wT_sb.append(w)
                s = phA.tile([P, BLK], BF16, name=f"xTs{kc}", tag=f"xTs{kc}")
                nc.sync.dma_start(s[:], xTs[kc * P:(kc + 1) * P, :])
                xTs_sb.append(s)

            # ---- rhs_i = qkvT[:, i_block] (bf16, SBUF) ------------------
            for dc in range(KC):
                ps = psum.tile([P, BLK], F32, name="ps", tag="ps")
                for kc in range(KC):
                    nc.tensor.matmul(
                        ps[:],
                        wT_sb[kc][:, dc * P:(dc + 1) * P],
                        xTs_sb[kc][:],
                        start=(kc == 0),
                        stop=(kc == KC - 1),
                    )
                rt = sb.tile([P, BLK], BF16, name=f"rhs_i{dc}", tag=f"rhs_i{dc}")
                nc.scalar.copy(rt[:], ps[:])
                rhs_i.append(rt)

            # ---- natural qkv shard -> AllGather -> qn_sb -----------------
            # qkv[i_block, :] = x[i_block, :] @ W.T, gathered across cores.
            ag_in = dram.tile([BLK, D], BF16, name="ag_in")
            ag_out2 = dram.tile([N, D], BF16, name="ag_out", addr_space="Shared")
            for jl in range(4):
                ps = psum.tile([P, D], F32, name="ps", tag="ps")
                for kc in range(KC):
                    nc.tensor.matmul(
                        ps[:],
                        xTs_sb[kc][:, jl * P:(jl + 1) * P],
                        wT_sb[kc][:],
                        start=(kc == 0),
                        stop=(kc == KC - 1),
                    )
                qs = phA.tile([P, D], BF16, name="qshard", tag="qshard", bufs=2)
                nc.scalar.copy(qs[:], ps[:])
                nc.sync.dma_start(ag_in[jl * P:(jl + 1) * P, :], qs[:])
            nc.gpsimd.collective_compute(
                "AllGather",
                mybir.AluOpType.bypass,
                replica_groups=[list(range(NC))],
                ins=[ag_in[:]],
                outs=[ag_out2[:]],
            )

            # ---- qkvT full -> SBUF bf16, interleaved with logits + exp +
            # ---- partial column sums + per-chunk |sigma| broadcast ------
            nz_tiles = {}
            pabs_tiles = {}
            for nch in range(NCH):
                # pace two noise-row loads + |i-j| tiles per n-chunk so the
                # P-path streams alongside the matmul pipeline
                k0 = nch * 2
                for k in (k0, k0 + 1):
                    ic, h = divmod(k, NH)
                    nz = pstream.tile([P, HW_], F32, name="nz", tag="nz", bufs=4)
                    nc.sync.dma_start(
                        nz[:], noise[ic * P:(ic + 1) * P, h * HW_:(h + 1) * HW_]
                    )
                    nz_tiles[(ic, h)] = nz
                    pabs = pstream.tile([P, HW_], F32, name="pabs", tag="pabs", bufs=4)
                    nc.gpsimd.iota(
                        pabs[:], pattern=[[-1, HW_]], base=ic * P - h * HW_,
                        channel_multiplier=1, allow_small_or_imprecise_dtypes=True,
                    )
                    nc.scalar.activation(
                        pabs[:], pabs[:], mybir.ActivationFunctionType.Abs,
                        bias=ioff_sb[:, 0:1],
                    )
                    pabs_tiles[(ic, h)] = pabs
                xTn = []
                for kc in range(KC):
                    t = phA.tile([P, 512], BF16, name="xTn", tag=f"xTn{kc}", bufs=3)
                    nc.sync.dma_start(
                        t[:], xT[kc * P:(kc + 1) * P, nch * 512:(nch + 1) * 512]
                    )
                    xTn.append(t)
                for dc in range(KC):
                    ps = psum.tile([P, 512], F32, name="ps", tag="ps")
                    for kc in range(KC):
                        nc.tensor.matmul(
                            ps[:],
                            wT_sb[kc][:, dc * P:(dc + 1) * P],
                            xTn[kc][:],
                            start=(kc == 0),
                            stop=(kc == KC - 1),
                        )
                    if dc % 2 == 0:
                        nc.scalar.copy(qkvT_sb[dc][:, nch * 512:(nch + 1) * 512], ps[:])
                    else:
                        nc.vector.tensor_copy(qkvT_sb[dc][:, nch * 512:(nch + 1) * 512], ps[:])
                # |sigma| broadcast chunk: row 0 of qkvT -> all 128 partitions
                sg = phA.tile([1, 512], BF16, name="sg", tag="sg", bufs=2)
                nc.scalar.activation(
                    sg[:], qkvT_sb[0][0:1, nch * 512:(nch + 1) * 512],
                    mybir.ActivationFunctionType.Abs,
                )
                pb = psum.tile([P, 512], F32, name="ps", tag="ps")
                nc.tensor.matmul(pb[:], ones[:], sg[:], start=True, stop=True)
                nc.vector.tensor_copy(sigbc[:, nch * 512:(nch + 1) * 512], pb[:])
                # logits for the 4 j-chunks covered by this n-chunk
                for jl in range(4):
                    jc = nch * 4 + jl
                    ps = psum.tile([P, BLK], F32, name="ps", tag="ps")
                    for dc in range(KC):
                        nc.tensor.matmul(
                            ps[:],
                            qkvT_sb[dc][:, jc * P:(jc + 1) * P],
                            rhs_i[dc][:],
                            start=(dc == 0), stop=(dc == KC - 1),
                        )
                    et = sb.tile([P, BLK], BF16, name=f"expT{jc}", tag=f"expT{jc}")
                    nc.scalar.activation(
                        et[:], ps[:], mybir.ActivationFunctionType.Exp,
                        scale=INV_SQRT_D, accum_out=pd[:, jc:jc + 1],
                    )
                    expT.append(et)


        cc_in = dram.tile([P, JC], F32, name="cc_in")
        cc_out = dram.tile([P, JC], F32, name="cc_out", addr_space="Shared")
        nc.sync.dma_start(cc_in[:], pd[:])
        pd_full = sb.tile([P, JC], F32, name="pd_full", tag="pd_full")
        rd = sb.tile([P, JC], F32, name="rd", tag="rd")

        # ---- qn readback before the P-output stream hogs the queue --
        for jc in range(JC):
            nc.sync.dma_start(qn_sb[jc][:], ag_out2[jc * P:(jc + 1) * P, :])

        # ---- P rows (emitted before the AllReduce: hides its latency)
        if True:
            for ic in range(IC):
                rs = sb.tile([P, NH], F32, name=f"rs{ic}", tag=f"rs{ic}")
                gaus = []
                for h in range(NH):
                    j0 = h * HW_
                    nz = nz_tiles[(ic, h)]
                    pabs = pabs_tiles[(ic, h)]
                    gau = pstream.tile([P, HW_], F32, name="gau", tag="gau", bufs=NH + 1)
                    nc.gpsimd.tensor_tensor(
                        gau[:], nz[:], sigbc[:, j0:j0 + HW_], mybir.AluOpType.mult
                    )
                    nc.vector.scalar_tensor_tensor(
                        gau[:], gau[:], 0.0, pabs[:],
                        op0=mybir.AluOpType.add, op1=mybir.AluOpType.add,
                        accum_out=rs[:, h:h + 1],
                    )
                    gaus.append(gau)
                rsum = sb.tile([P, 1], F32, name=f"rsum{ic}", tag=f"rsum{ic}")
                nc.vector.tensor_reduce(
                    rsum[:], rs[:], axis=mybir.AxisListType.X, op=mybir.AluOpType.add
                )
                rr = sb.tile([P, 1], F32, name=f"rr{ic}", tag=f"rr{ic}")
                nc.vector.reciprocal(rr[:], rsum[:])
                for h in range(NH):
                    j0 = h * HW_
                    nc.vector.tensor_scalar_mul(gaus[h][:], gaus[h][:], rr[:, 0:1])
                    nc.sync.dma_start(
                        out_p[ic * P:(ic + 1) * P, j0:j0 + HW_], gaus[h][:]
                    )

        # ---- AllReduce the softmax denominators ---------------------
        nc.gpsimd.collective_compute(
            "AllReduce",
            mybir.AluOpType.add,
            replica_groups=[list(range(NC))],
            ins=[cc_in[:]],
            outs=[cc_out[:]],
        )
        nc.sync.dma_start(pd_full[:], cc_out[:])
        nc.vector.reciprocal(rd[:], pd_full[:])

        # ---- normalize S.T in place (scalar engine: Copy with scale) -
        for jc in range(JC):
            nc.scalar.activation(
                expT[jc][:], expT[jc][:], mybir.ActivationFunctionType.Copy,
                scale=rd[:, jc:jc + 1],
            )

        # ---- Z.T block: 4 PSUM banks accumulate in parallel ---------
        psz = [
            psum.tile([P, BLK], F32, name=f"psz{dc}", tag=f"psz{dc}", bufs=1)
            for dc in range(KC)
        ]
        for jc in range(JC):
            for dc in range(KC):
                nc.tensor.matmul(
                    psz[dc][:], qn_sb[jc][:, dc * P:(dc + 1) * P], expT[jc][:],
                    start=(jc == 0), stop=(jc == JC - 1),
                )
        with tc.tile_pool(name="zout", bufs=2) as zout:
            for dc in range(KC):
                zt = zout.tile([P, BLK], F32, name="zt_cp", tag="zt_cp")
                nc.scalar.copy(zt[:], psz[dc][:])
                nc.sync.dma_start(out_zt[dc * P:(dc + 1) * P, :], zt[:])

    nc.compile()
    return nc


def _get_compiled():
    global _compiled
    if _compiled is None:
        _compiled = _build()
    return _compiled


def _make_noise():
    import jax
    import jax.numpy as jnp

    return np.asarray(
        jax.random.normal(jax.random.key(42), (N, N), dtype=jnp.float32)
    )


def make_in_maps(x, W, noise):
    bf = ml_dtypes.bfloat16
    xT = np.ascontiguousarray(x.T.astype(bf))
    wT = np.ascontiguousarray(W.T.astype(bf))
    in_maps = []
    for c in range(NC):
        in_maps.append({
            "xT": xT,
            "xTs": np.ascontiguousarray(xT[:, c * BLK:(c + 1) * BLK]),
            "wT": wT,
            "noise": np.ascontiguousarray(noise[c * BLK:(c + 1) * BLK, :]),
            "ioff": np.full((P, 1), c * BLK, dtype=np.float32),
        })
    return in_maps


def assemble(results):
    Z = np.concatenate([results[c]["zt"].T for c in range(NC)], axis=0)
    Pm = np.concatenate([results[c]["p"] for c in range(NC)], axis=0)
    return Z, Pm


def kernel(x, W):
    x = np.ascontiguousarray(np.asarray(x, dtype=np.float32))
    W = np.ascontiguousarray(np.asarray(W, dtype=np.float32))
    noise = _make_noise()
    nc = _get_compiled()
    in_maps = make_in_maps(x, W, noise)
    res = bass_utils.run_bass_kernel_spmd(
        nc, in_maps, core_ids=list(range(NC)), trace=False
    )
    return assemble(res.results)


# revision 21
# speedup vs baseline: 1.2998x; 1.2998x over previous
"""AnomalyAttention distributed Bass kernel for 8 TRN2 NeuronCores.

Reference computation (n=4096, d=512):
    qkv = x @ W.T                       # [n, d];  Q = K = V = sigma = qkv
    L   = (Q @ K.T) / sqrt(d)           # [n, n]
    S   = softmax(L, axis=0)            # column softmax
    Z   = S @ V                         # [n, d]
    p[i,j]    = |i - j|
    gaussian  = p + |sigma[:,0]|[None,:] * noise      # noise = fixed jax key(42)
    P   = gaussian / gaussian.sum(-1, keepdims=True)  # row normalized
    returns (Z, P)

Sharding: each core owns a 512-row block i_block = [c*512, (c+1)*512).
Logits are built transposed, L.T[j, i_local] (all j on partitions, local i on
free), so the column softmax reduces along the free axis per partition; the
cross-core part of the reduction is a single 16 KiB AllReduce of per-column
partial sums.  The natural-layout qkv needed by Z.T = qkv.T @ S.T comes from
an early AllGather of each core's locally-computed shard; Z streams its lhsT
tiles straight from the gathered DRAM buffer.

The P-path (prior matrix) is software-pipelined INTO the main loop two tiles
per n-chunk in h-major order, so each gaussian tile's |sigma| broadcast
columns are ready just in time; its row sums complete right as the main loop
ends, the 1/rowsum scales and stores fill the AllReduce latency, and the Z
matmuls stream behind the scalar-engine softmax normalization.
"""

import sys

if "/opt/trn_rl_repo" not in sys.path:
    sys.path.insert(0, "/opt/trn_rl_repo")

from contextlib import ExitStack

import ml_dtypes
import numpy as np

import concourse.bass as bass
import concourse.tile as tile
from concourse import bacc, mybir, bass_utils

N = 4096
D = 512
NC = 8
BLK = N // NC          # 512 rows of S / P per core
P = 128                # partitions
F32 = mybir.dt.float32
BF16 = mybir.dt.bfloat16
INV_SQRT_D = 1.0 / np.sqrt(D)

KC = D // P            # 4 contraction chunks of 128
JC = N // P            # 32 j-chunks of 128
NCH = N // 512         # 8 n-chunks of 512
IC = BLK // P          # 4 local i-chunks of 128
HW_ = 1024             # P-path free-dim tile
NH = N // HW_          # free chunks per i-chunk row (4)
NPK = IC * NH          # 16 P-path tiles, processed h-major: k = h*IC + ic

_compiled = None


def _build():
    nc = bacc.Bacc("TRN2", target_bir_lowering=False, debug=False, num_devices=NC)

    xT = nc.dram_tensor("xT", [D, N], BF16, kind="ExternalInput").ap()      # x.T
    xTs = nc.dram_tensor("xTs", [D, BLK], BF16, kind="ExternalInput").ap()  # x.T[:, i_block]
    wT = nc.dram_tensor("wT", [D, D], BF16, kind="ExternalInput").ap()      # W.T
    noise = nc.dram_tensor("noise", [BLK, N], BF16, kind="ExternalInput").ap()
    ioff = nc.dram_tensor("ioff", [P, 1], F32, kind="ExternalInput").ap()   # c*BLK

    out_zt = nc.dram_tensor("zt", [D, BLK], F32, kind="ExternalOutput").ap()  # Z.T block
    out_p = nc.dram_tensor("p", [BLK, N], F32, kind="ExternalOutput").ap()    # P rows

    with tile.TileContext(nc) as tc, ExitStack() as big:
        sb = big.enter_context(tc.tile_pool(name="sb", bufs=1))
        psum = big.enter_context(tc.tile_pool(name="psum", bufs=4, space="PSUM"))
        dram = big.enter_context(tc.tile_pool(name="dram", bufs=1, space="DRAM"))
        pstream = big.enter_context(tc.tile_pool(name="pstream", bufs=1))

        qkvT_sb = [
            sb.tile([P, N], BF16, name=f"qkvT{dc}", tag=f"qkvT{dc}") for dc in range(KC)
        ]
        sigbc = sb.tile([P, N], BF16, name="sigbc", tag="sigbc")
        ones = sb.tile([1, P], BF16, name="ones", tag="ones")
        nc.vector.memset(ones[:], 1.0)
        ioff_sb = sb.tile([P, 1], F32, name="ioff_sb", tag="ioff_sb")
        nc.sync.dma_start(ioff_sb[:], ioff[:])
        pd = sb.tile([P, JC], F32, name="pd", tag="pd")
        rs = sb.tile([P, NH * IC], F32, name="rs", tag="rs")  # col = h*IC+ic
        rhs_i = []
        expT = []
        # P-path pipeline state
    # (k = h*IC + ic), stt lags its tt by two n-chunks to keep DVE flowing
        gau_tiles = {}
        nz_tiles = {}
        pabs_tiles = {}

        def emit_p_front(k):
            """nz load + iota + |.| + sigma*noise for P tile k (h-major)."""
            h, ic = divmod(k, IC)
            j0 = h * HW_
            nz = pstream.tile([P, HW_], BF16, name="nz", tag="nz", bufs=3)
            nc.sync.dma_start(nz[:], noise[ic * P:(ic + 1) * P, j0:j0 + HW_])
            nz_tiles[k] = nz
            pabs = pstream.tile([P, HW_], F32, name="pabs", tag="pabs", bufs=3)
            nc.gpsimd.iota(
                pabs[:], pattern=[[-1, HW_]], base=ic * P - j0,
                channel_multiplier=1, allow_small_or_imprecise_dtypes=True,
            )
            nc.scalar.activation(
                pabs[:], pabs[:], mybir.ActivationFunctionType.Abs,
                bias=ioff_sb[:, 0:1],
            )
            pabs_tiles[k] = pabs
            gau = pstream.tile([P, HW_], F32, name="gau", tag="gau", bufs=NPK)
            nc.gpsimd.tensor_tensor(
                gau[:], nz[:], sigbc[:, j0:j0 + HW_], mybir.AluOpType.mult
            )
            gau_tiles[k] = gau

        def emit_p_mid(k):
            """gaussian = sigma*noise + |i-j|, with fused row-sum accum."""
            gau = gau_tiles[k]
            nc.vector.scalar_tensor_tensor(
                gau[:], gau[:], 0.0, pabs_tiles[k][:],
                op0=mybir.AluOpType.add, op1=mybir.AluOpType.add,
                accum_out=rs[:, k:k + 1],
            )

        with tc.tile_pool(name="phA", bufs=1) as phA:
            # ---- load W.T, x.T slice ------------------------------------
            wT_sb, xTs_sb = [], []
            for kc in range(KC):
                w = phA.tile([P, D], BF16, name=f"wT{kc}", tag=f"wT{kc}")
                nc.sync.dma_start(w[:], wT[kc * P:(kc + 1) * P, :])
                wT_sb.append(w)
                s = phA.tile([P, BLK], BF16, name=f"xTs{kc}", tag=f"xTs{kc}")
                nc.sync.dma_start(s[:], xTs[kc * P:(kc + 1) * P, :])
                xTs_sb.append(s)

            # ---- rhs_i = qkvT[:, i_block] (bf16, SBUF) ------------------
            for dc in range(KC):
                ps = psum.tile([P, BLK], F32, name="ps", tag="ps")
                for kc in range(KC):
                    nc.tensor.matmul(
                        ps[:],
                        wT_sb[kc][:, dc * P:(dc + 1) * P],
                        xTs_sb[kc][:],
                        start=(kc == 0),
                        stop=(kc == KC - 1),
                    )
                rt = sb.tile([P, BLK], BF16, name=f"rhs_i{dc}", tag=f"rhs_i{dc}")
                nc.scalar.copy(rt[:], ps[:])
                rhs_i.append(rt)

            # ---- natural qkv shard -> AllGather -------------------------
            ag_in = dram.tile([BLK, D], BF16, name="ag_in")
            ag_out = dram.tile([N, D], BF16, name="ag_out", addr_space="Shared")
            for jl in range(4):
                ps = psum.tile([P, D], F32, name="ps", tag="ps")
                for kc in range(KC):
                    nc.tensor.matmul(
                        ps[:],
                        xTs_sb[kc][:, jl * P:(jl + 1) * P],
                        wT_sb[kc][:],
                        start=(kc == 0),
                        stop=(kc == KC - 1),
                    )
                qs = phA.tile([P, D], BF16, name="qshard", tag="qshard", bufs=2)
                nc.scalar.copy(qs[:], ps[:])
                nc.sync.dma_start(ag_in[jl * P:(jl + 1) * P, :], qs[:])
            nc.gpsimd.collective_compute(
                "AllGather",
                mybir.AluOpType.bypass,
                replica_groups=[list(range(NC))],
                ins=[ag_in[:]],
                outs=[ag_out[:]],
            )

            # ---- main loop: qkvT chunks + sigma bcast + logits + exp,
            # ---- with the P-path pipelined in at 2 tiles per n-chunk ----
            for nch in range(NCH):
                if nch >= 1:
                    emit_p_front(2 * (nch - 1))
                xTn = []
                for kc in range(KC):
                    t = phA.tile([P, 512], BF16, name="xTn", tag=f"xTn{kc}", bufs=2)
                    nc.sync.dma_start(
                        t[:], xT[kc * P:(kc + 1) * P, nch * 512:(nch + 1) * 512]
                    )
                    xTn.append(t)
                for dc in range(KC):
                    ps = psum.tile([P, 512], F32, name="ps", tag="ps")
                    for kc in range(KC):
                        nc.tensor.matmul(
                            ps[:],
                            wT_sb[kc][:, dc * P:(dc + 1) * P],
                            xTn[kc][:],
                            start=(kc == 0),
                            stop=(kc == KC - 1),
                        )
                    if dc % 2 == 0:
                        nc.scalar.copy(qkvT_sb[dc][:, nch * 512:(nch + 1) * 512], ps[:])
                    else:
                        nc.vector.tensor_copy(
                            qkvT_sb[dc][:, nch * 512:(nch + 1) * 512], ps[:]
                        )
                # |sigma| broadcast chunk: row 0 of qkvT -> all partitions
                sg = phA.tile([1, 512], BF16, name="sg", tag="sg", bufs=2)
                nc.scalar.activation(
                    sg[:], qkvT_sb[0][0:1, nch * 512:(nch + 1) * 512],
                    mybir.ActivationFunctionType.Abs,
                )
                pb = psum.tile([P, 512], F32, name="ps", tag="ps")
                nc.tensor.matmul(pb[:], ones[:], sg[:], start=True, stop=True)
                nc.vector.tensor_copy(sigbc[:, nch * 512:(nch + 1) * 512], pb[:])
                if nch >= 1:
                    emit_p_front(2 * (nch - 1) + 1)
                # P-path tails, two n-chunks behind their fronts
                if nch >= 3:
                    emit_p_mid(2 * (nch - 3))
                    emit_p_mid(2 * (nch - 3) + 1)
                # logits for the 4 j-chunks covered by this n-chunk
                for jl in range(4):
                    jc = nch * 4 + jl
                    ps = psum.tile([P, BLK], F32, name="ps", tag="ps")
                    for dc in range(KC):
                        nc.tensor.matmul(
                            ps[:],
                            qkvT_sb[dc][:, jc * P:(jc + 1) * P],
                            rhs_i[dc][:],
                            start=(dc == 0), stop=(dc == KC - 1),
                        )
                    et = sb.tile([P, BLK], BF16, name=f"expT{jc}", tag=f"expT{jc}")
                    nc.scalar.activation(
                        et[:], ps[:], mybir.ActivationFunctionType.Exp,
                        scale=INV_SQRT_D, accum_out=pd[:, jc:jc + 1],
                    )
                    expT.append(et)

        # ---- AllReduce the softmax denominators ---------------------
        cc_in = dram.tile([P, JC], F32, name="cc_in")
        cc_out = dram.tile([P, JC], F32, name="cc_out", addr_space="Shared")
        nc.sync.dma_start(cc_in[:], pd[:])
        nc.gpsimd.collective_compute(
            "AllReduce",
            mybir.AluOpType.add,
            replica_groups=[list(range(NC))],
            ins=[cc_in[:]],
            outs=[cc_out[:]],
        )
        pd_full = sb.tile([P, JC], F32, name="pd_full", tag="pd_full")
        rd = sb.tile([P, JC], F32, name="rd", tag="rd")

        # ---- P-path drain: last fronts + remaining row-sum tails ----
        emit_p_front(NPK - 2)
        emit_p_front(NPK - 1)
        for k in range(NPK - 6, NPK):
            emit_p_mid(k)

        # ---- P rows: scale by 1/rowsum and store --------------------
        for ic in range(IC):
            rsum = sb.tile([P, 1], F32, name=f"rsum{ic}", tag=f"rsum{ic}")
            nc.vector.tensor_reduce(
                rsum[:], rs[:, ic:NPK:IC], axis=mybir.AxisListType.X,
                op=mybir.AluOpType.add,
            )
            rr = sb.tile([P, 1], F32, name=f"rr{ic}", tag=f"rr{ic}")
            nc.vector.reciprocal(rr[:], rsum[:])
            for h in range(NH):
                k = h * IC + ic
                gau = gau_tiles[k]
                nc.vector.tensor_scalar_mul(gau[:], gau[:], rr[:, 0:1])
                nc.sync.dma_start(
                    out_p[ic * P:(ic + 1) * P, h * HW_:(h + 1) * HW_], gau[:]
                )

        # ---- softmax denominators: readback + reciprocal ------------
        nc.scalar.dma_start(pd_full[:], cc_out[:])
        nc.vector.reciprocal(rd[:], pd_full[:])

        # ---- normalize S.T in place (scalar engine) -----------------
        for jc in range(JC):
            nc.scalar.activation(
                expT[jc][:], expT[jc][:], mybir.ActivationFunctionType.Copy,
                scale=rd[:, jc:jc + 1],
            )

        # ---- Z.T block: 4 PSUM banks accumulate in parallel ---------
        psz = [
            psum.tile([P, BLK], F32, name=f"psz{dc}", tag=f"psz{dc}", bufs=1)
            for dc in range(KC)
        ]
        with tc.tile_pool(name="zstream", bufs=4) as zstream:
            for jc in range(JC):
                qn = zstream.tile([P, D], BF16, name="z_qn", tag="z_qn")
                nc.sync.dma_start(qn[:], ag_out[jc * P:(jc + 1) * P, :])
                for dc in range(KC):
                    nc.tensor.matmul(
                        psz[dc][:], qn[:, dc * P:(dc + 1) * P], expT[jc][:],
                        start=(jc == 0), stop=(jc == JC - 1),
                    )
            for dc in range(KC):
                zt = zstream.tile([P, BLK], F32, name="zt_cp", tag="zt_cp", bufs=2)
                nc.scalar.copy(zt[:], psz[dc][:])
                nc.sync.dma_start(out_zt[dc * P:(dc + 1) * P, :], zt[:])

    nc.compile()
    return nc


def _get_compiled():
    global _compiled
    if _compiled is None:
        _compiled = _build()
    return _compiled


def _make_noise():
    import jax
    import jax.numpy as jnp

    return np.asarray(
        jax.random.normal(jax.random.key(42), (N, N), dtype=jnp.float32)
    )


def make_in_maps(x, W, noise):
    bf = ml_dtypes.bfloat16
    xT = np.ascontiguousarray(x.T.astype(bf))
    wT = np.ascontiguousarray(W.T.astype(bf))
    noise_bf = noise.astype(bf)
    in_maps = []
    for c in range(NC):
        in_maps.append({
            "xT": xT,
            "xTs": np.ascontiguousarray(xT[:, c * BLK:(c + 1) * BLK]),
            "wT": wT,
            "noise": np.ascontiguousarray(noise_bf[c * BLK:(c + 1) * BLK, :]),
            "ioff": np.full((P, 1), c * BLK, dtype=np.float32),
        })
    return in_maps


def assemble(results):
    Z = np.concatenate([results[c]["zt"].T for c in range(NC)], axis=0)
    Pm = np.concatenate([results[c]["p"] for c in range(NC)], axis=0)
    return Z, Pm


def kernel(x, W):
    x = np.ascontiguousarray(np.asarray(x, dtype=np.float32))
    W = np.ascontiguousarray(np.asarray(W, dtype=np.float32))
    noise = _make_noise()
    nc = _get_compiled()
    in_maps = make_in_maps(x, W, noise)
    last_err = None
    for _ in range(3):
        try:
            res = bass_utils.run_bass_kernel_spmd(
                nc, in_maps, core_ids=list(range(NC)), trace=False
            )
            return assemble(res.results)
        except Exception as e:  # transient NRT/device hiccups: retry
            last_err = e
    raise last_err
